# revision 13
# baseline (speedup 1.0000x reference)
"""Bond-aware message passing GNN kernel for 8 Trainium2 NeuronCores.

Strategy (edge-parallel, col-sorted):
  - Host: sort edges by destination (col), shard contiguous ranges of sorted
    edges across 8 cores, pack per-512-edge-tile inputs in feature-major
    (transposed) layout so the device only does W-stationary matmuls.
  - Device per 512-edge tile:
      L1:  h_m = W1_m.T @ combinedT  (3 MLPs, K=145 split 128+17), PSUM
      act: h_m = silu(h_m + b1_m)    (ScalarE, bias fused)
      L2:  msgT = W2cat.T @ h        (3 matmuls into one PSUM tile)
      bias+move to SBUF, PE-transpose payload to edge-major,
      selection-matrix scatter matmul (segment-sum within the tile's
      64-node window), window results stacked to DRAM.
  - Host: overlap-add the per-tile 64-node windows into the final
    aggregated_x / aggregated_pos; inverse-permute edge_update.

No collectives: cores own disjoint edge ranges; window overlap across
tile/core boundaries is resolved in the host merge.
"""

import os

import numpy as np

import concourse.bass as bass
from concourse import bacc
import concourse.mybir as mybir
import concourse.tile as tile
from concourse import bass_utils
from concourse.masks import make_identity

F32 = mybir.dt.float32
I32 = mybir.dt.int32

# model dims (fixed by the problem)
IN_D, BOND_D, HID, OUT_D = 64, 16, 128, 64
COMB = 2 * IN_D + BOND_D + 1  # 145
KLO = COMB - 128  # 17
NCORES = 8
TILE = 512  # edges per device tile
SUB = 128  # edges per subtile (matmul K)
WIN = 64  # node-window slots per tile
PAY = 67  # payload rows: 64 msg_x + 3 pos_update
# P2 psum row layout: [0:64] msg_x, [64:67] w_pos(x3), [96:112] edge_update
EU_LO, EU_HI = 96, 112

LAST_RESULTS = None  # BassKernelResults of the most recent run (for test.py)
ACT_FUNC = mybir.ActivationFunctionType.Silu  # overridable for CoreSim tests


# ----------------------------------------------------------------------------
# host-side packing
# ----------------------------------------------------------------------------

def _plan_tiles(col_s, lo, hi):
    """Greedy tiling of sorted edge range [lo,hi): each tile takes <=TILE edges
    whose cols stay within a WIN-node window. Returns list of (pos, take, base)."""
    tiles = []
    pos = lo
    while pos < hi:
        base = int(col_s[pos])
        end = min(pos + TILE, hi)
        take = int(np.searchsorted(col_s[pos:end], base + WIN, side="left"))
        assert take > 0
        tiles.append((pos, take, base))
        pos += take
    return tiles


def _pack_core(x, pos_arr, ea_s, row_s, col_s, rel_s, dsq_s, lo, hi, T):
    """Build device input arrays for one core's sorted-edge range [lo, hi)."""
    tiles = _plan_tiles(col_s, lo, hi)
    assert len(tiles) <= T
    E_pad = T * TILE

    # per-slot source index into the sorted arrays (-1 = padding)
    idx = np.full((T, TILE), -1, dtype=np.int64)
    bases = np.zeros((T,), dtype=np.int64)
    for t, (p, take, b) in enumerate(tiles):
        idx[t, :take] = np.arange(p, p + take)
        bases[t] = b
    valid = idx >= 0
    cidx = np.where(valid, idx, 0)

    r = np.where(valid, row_s[cidx], 0)
    c = np.where(valid, col_s[cidx], 0)

    xr = x[r]  # [T, TILE, 64]
    xc = x[c]
    ea = ea_s[cidx]  # [T, TILE, 16]
    rel = rel_s[cidx]  # [T, TILE, 3]
    dsq = dsq_s[cidx]  # [T, TILE]
    m = valid[..., None]
    xr = np.where(m, xr, 0.0)
    xc = np.where(m, xc, 0.0)
    ea = np.where(m, ea, 0.0)
    rel = np.where(m, rel, 0.0)
    dsq = np.where(valid, dsq, 0.0)

    comb_hi = np.concatenate(
        [xr.transpose(0, 2, 1), xc.transpose(0, 2, 1)], axis=1
    ).astype(np.float32)  # [T, 128, TILE]
    comb_lo = np.concatenate(
        [ea.transpose(0, 2, 1), dsq[:, None, :]], axis=1
    ).astype(np.float32)  # [T, 17, TILE]

    # edge-major per-subtile tensors: [T, 128, TILE//SUB, k]
    nsub = TILE // SUB
    rel_em = (
        rel.reshape(T, nsub, SUB, 3).transpose(0, 2, 1, 3).astype(np.float32)
    )  # [T, 128, 4, 3]
    col_local = np.where(valid, c - bases[:, None], -1).astype(np.float32)
    colw = col_local.reshape(T, nsub, SUB).transpose(0, 2, 1).astype(np.float32)
    colw = np.ascontiguousarray(colw)  # [T, 128, 4]

    return {
        "comb_hi": comb_hi,
        "comb_lo": np.ascontiguousarray(comb_lo),
        "rel_em": np.ascontiguousarray(rel_em),
        "colw": colw,
    }, idx, bases


def _pack_weights(W1x, W1p, W1e, b1x, b1p, b1e, W2x, W2p, W2e, b2x, b2p, b2e):
    W1hi = np.stack([W1x[:128], W1p[:128], W1e[:128]], axis=1)  # [128, 3, HID]
    W1lo = np.stack([W1x[128:], W1p[128:], W1e[128:]], axis=1)  # [17, 3, HID]
    b1 = np.stack([b1x, b1p, b1e], axis=1)  # [HID, 3]
    W2cat = np.concatenate(
        [W2x, np.repeat(W2p, 3, axis=1), W2e], axis=1
    )  # [HID, 64+3+16=83]
    b2cat = np.zeros((EU_HI, 1), np.float32)
    b2cat[0:64, 0] = b2x
    b2cat[64:67, 0] = b2p
    b2cat[EU_LO:EU_HI, 0] = b2e
    return {
        "W1hi": np.ascontiguousarray(W1hi, np.float32),
        "W1lo": np.ascontiguousarray(W1lo, np.float32),
        "b1": np.ascontiguousarray(b1, np.float32),
        "W2cat": np.ascontiguousarray(W2cat, np.float32),
        "b2cat": b2cat,
    }


# ----------------------------------------------------------------------------
# device program
# ----------------------------------------------------------------------------

def _build_bass(T):
    nc = bacc.Bacc(trn_type="TRN2")
    nsub = TILE // SUB

    d_chi = nc.dram_tensor("comb_hi", [T, 128, TILE], F32, kind="ExternalInput")
    d_clo = nc.dram_tensor("comb_lo", [T, KLO, TILE], F32, kind="ExternalInput")
    d_rel = nc.dram_tensor("rel_em", [T, 128, nsub, 3], F32, kind="ExternalInput")
    d_colw = nc.dram_tensor("colw", [T, 128, nsub], F32, kind="ExternalInput")
    d_W1hi = nc.dram_tensor("W1hi", [128, 3, HID], F32, kind="ExternalInput")
    d_W1lo = nc.dram_tensor("W1lo", [KLO, 3, HID], F32, kind="ExternalInput")
    d_b1 = nc.dram_tensor("b1", [HID, 3], F32, kind="ExternalInput")
    d_W2cat = nc.dram_tensor("W2cat", [HID, 83], F32, kind="ExternalInput")
    d_b2cat = nc.dram_tensor("b2cat", [EU_HI, 1], F32, kind="ExternalInput")

    d_win = nc.dram_tensor("winstack", [WIN, T * PAY], F32, kind="ExternalOutput")
    d_eu = nc.dram_tensor("euT", [BOND_D, T * TILE], F32, kind="ExternalOutput")

    with tile.TileContext(nc) as tc:
        with (
            tc.tile_pool(name="const", bufs=1) as constp,
            tc.tile_pool(name="wts", bufs=1) as wp,
            tc.tile_pool(name="io", bufs=3) as iop,
            tc.tile_pool(name="work", bufs=2) as workp,
            tc.tile_pool(name="winsb", bufs=1) as winp,
            tc.tile_pool(name="ph", bufs=1, space="PSUM") as php,
            tc.tile_pool(name="p2", bufs=2, space="PSUM") as p2p,
            tc.tile_pool(name="ptr", bufs=2, space="PSUM") as ptrp,
            tc.tile_pool(name="pagg", bufs=1, space="PSUM") as paggp,
        ):
            # ---- constants / weights (loaded once) ----
            ident = constp.tile([128, 128], F32)
            make_identity(nc, ident[:])

            iota_i = constp.tile([128, nsub, WIN], I32)
            nc.gpsimd.iota(iota_i[:], pattern=[[0, nsub], [1, WIN]], base=0,
                           channel_multiplier=0)
            iota_f = constp.tile([128, nsub, WIN], F32)
            nc.vector.tensor_copy(iota_f[:], iota_i[:])

            w1hi = wp.tile([128, 3, HID], F32)
            nc.sync.dma_start(out=w1hi[:], in_=d_W1hi[:])
            w1lo = wp.tile([KLO, 3, HID], F32)
            nc.sync.dma_start(out=w1lo[:], in_=d_W1lo[:])
            b1 = wp.tile([HID, 3], F32)
            nc.sync.dma_start(out=b1[:], in_=d_b1[:])
            w2cat = wp.tile([HID, 83], F32)
            nc.sync.dma_start(out=w2cat[:], in_=d_W2cat[:])
            b2cat = wp.tile([EU_HI, 1], F32)
            nc.sync.dma_start(out=b2cat[:], in_=d_b2cat[:])

            # winstack staging buffer in SBUF (one DMA at the end); slot-major
            win_sb = winp.tile([WIN, T * PAY], F32)

            for t in range(T):
                chi = iop.tile([128, TILE], F32, tag="chi")
                nc.sync.dma_start(out=chi[:], in_=d_chi[t])
                clo = iop.tile([KLO, TILE], F32, tag="clo")
                nc.sync.dma_start(out=clo[:], in_=d_clo[t])
                rel = iop.tile([128, nsub, 3], F32, tag="rel")
                nc.sync.dma_start(out=rel[:], in_=d_rel[t])
                colw = iop.tile([128, nsub], F32, tag="colw")
                nc.sync.dma_start(out=colw[:], in_=d_colw[t])

                # ---- L1: h[m] = W1_m.T @ combT ----
                h_ps = php.tile([128, 3, TILE], F32, tag="hps")
                for m_ in range(3):
                    nc.tensor.matmul(
                        h_ps[:, m_, :], w1hi[:, m_, :], chi[:],
                        start=True, stop=False,
                    )
                    nc.tensor.matmul(
                        h_ps[:, m_, :], w1lo[:, m_, :], clo[:],
                        start=False, stop=True,
                    )

                # ---- silu(h + b1) -> SBUF ----
                h_sb = workp.tile([128, 3, TILE], F32, tag="hsb")
                for m_ in range(3):
                    nc.scalar.activation(
                        h_sb[:, m_, :], h_ps[:, m_, :],
                        ACT_FUNC,
                        bias=b1[:, m_ : m_ + 1],
                    )

                # ---- L2 into one PSUM tile ----
                p2 = p2p.tile([128, TILE], F32, tag="p2")
                nc.tensor.matmul(p2[0:64, :], w2cat[:, 0:64], h_sb[:, 0, :],
                                 start=True, stop=True)
                nc.tensor.matmul(p2[64:67, :], w2cat[:, 64:67], h_sb[:, 1, :],
                                 start=True, stop=True, tile_position=(0, 64))
                nc.tensor.matmul(p2[EU_LO:EU_HI, :], w2cat[:, 67:83], h_sb[:, 2, :],
                                 start=True, stop=True, tile_position=(0, EU_LO))

                # ---- +b2, move to SBUF (rows 67..95 are junk, never used) ----
                payT = workp.tile([EU_HI, TILE], F32, tag="payT")
                nc.vector.tensor_scalar(
                    out=payT[0:PAY, :], in0=p2[0:PAY, :],
                    scalar1=b2cat[0:PAY, 0:1], scalar2=None,
                    op0=mybir.AluOpType.add,
                )
                nc.vector.tensor_scalar(
                    out=payT[EU_LO:EU_HI, :], in0=p2[EU_LO:EU_HI, :],
                    scalar1=b2cat[EU_LO:EU_HI, 0:1], scalar2=None,
                    op0=mybir.AluOpType.add,
                )

                # ---- edge_update out ----
                nc.sync.dma_start(
                    out=d_eu[:, t * TILE : (t + 1) * TILE],
                    in_=payT[EU_LO:EU_HI, :],
                )

                # ---- transpose payload to edge-major ----
                peT = ptrp.tile([128, nsub, PAY], F32, tag="peT")
                for s in range(nsub):
                    nc.tensor.transpose(
                        peT[:, s, :],
                        payT[0:PAY, s * SUB : (s + 1) * SUB],
                        ident[0:PAY, 0:PAY],
                    )
                pay_em = workp.tile([128, nsub, PAY], F32, tag="payem")
                nc.vector.tensor_copy(pay_em[:], peT[:])
                # pos_update = w_pos * rel_pos
                nc.vector.tensor_tensor(
                    out=pay_em[:, :, 64:67], in0=pay_em[:, :, 64:67],
                    in1=rel[:], op=mybir.AluOpType.mult,
                )

                # ---- selection matrix S[e, slot] = (col_local[e] == slot) ----
                S = workp.tile([128, nsub, WIN], F32, tag="S")
                nc.vector.tensor_tensor(
                    out=S[:], in0=iota_f[:],
                    in1=colw[:, :, None].to_broadcast([128, nsub, WIN]),
                    op=mybir.AluOpType.is_equal,
                )

                # ---- scatter: agg[slot, feat] += S_s.T @ pay_s ----
                agg = paggp.tile([WIN, PAY], F32, tag="agg")
                for s in range(nsub):
                    nc.tensor.matmul(
                        agg[:, :], S[:, s, :], pay_em[:, s, :],
                        start=(s == 0), stop=(s == nsub - 1),
                    )
                # agg is [slot, feat]; store slot-major, host transposes
                nc.vector.tensor_copy(
                    win_sb[:, t * PAY : (t + 1) * PAY], agg[:]
                )

            nc.sync.dma_start(out=d_win[:], in_=win_sb[:])

    nc.finalize()
    return nc


# ----------------------------------------------------------------------------
# entry point
# ----------------------------------------------------------------------------

def host_pack(x, pos, edge_attr, edge_index,
              W1x, b1x, W2x, b2x,
              W1p, b1p, W2p, b2p,
              W1e, b1e, W2e, b2e):
    """Sort/shard/pack inputs. Returns (core_inputs, meta) where meta has
    everything host_merge needs."""
    x = np.asarray(x, np.float32)
    pos = np.asarray(pos, np.float32)
    edge_attr = np.asarray(edge_attr, np.float32)
    edge_index = np.asarray(edge_index)
    E = edge_index.shape[1]
    N = x.shape[0]

    row = edge_index[0].astype(np.int64)
    col = edge_index[1].astype(np.int64)
    order = np.argsort(col, kind="stable")
    row_s = row[order]
    col_s = col[order]
    ea_s = edge_attr[order]
    rel_s = pos[row_s] - pos[col_s]  # [E, 3]
    dsq_s = np.sum(rel_s * rel_s, axis=1)  # [E]

    # shard sorted edges evenly
    bounds = [E * c // NCORES for c in range(NCORES + 1)]
    plans = [
        _plan_tiles(col_s, bounds[c], bounds[c + 1]) for c in range(NCORES)
    ]
    T = max(len(p) for p in plans)

    core_inputs = []
    core_idx = []
    core_bases = []
    wts = _pack_weights(W1x, W1p, W1e, b1x, b1p, b1e,
                        W2x, W2p, W2e, b2x, b2p, b2e)
    for c in range(NCORES):
        inp, idx, bases = _pack_core(
            x, pos, ea_s, row_s, col_s, rel_s, dsq_s,
            bounds[c], bounds[c + 1], T,
        )
        inp.update(wts)
        core_inputs.append(inp)
        core_idx.append(idx)
        core_bases.append(bases)
    meta = {
        "N": N, "E": E, "T": T, "order": order, "plans": plans,
        "core_idx": core_idx, "core_bases": core_bases,
    }
    return core_inputs, meta


def host_merge(results, meta):
    N, E, T = meta["N"], meta["E"], meta["T"]
    order, plans = meta["order"], meta["plans"]
    core_idx, core_bases = meta["core_idx"], meta["core_bases"]

    aggX = np.zeros((N + WIN, OUT_D), np.float64)
    aggP = np.zeros((N + WIN, 3), np.float64)
    edge_update_s = np.empty((E, BOND_D), np.float32)
    for c in range(NCORES):
        out = results[c]
        win = out["winstack"].reshape(WIN, T, PAY)  # [slot, tile, feat]
        euT = out["euT"]  # [16, T*TILE]
        idx = core_idx[c]
        bases = core_bases[c]
        nt = len(plans[c])
        # overlap-add the windows
        wx = win[:, :nt, 0:64].transpose(1, 0, 2)  # [nt, WIN, 64]
        wpp = win[:, :nt, 64:67].transpose(1, 0, 2)  # [nt, WIN, 3]
        tgt = (bases[:nt, None] + np.arange(WIN)[None, :]).ravel()
        np.add.at(aggX, tgt, wx.reshape(-1, OUT_D))
        np.add.at(aggP, tgt, wpp.reshape(-1, 3))
        # edge updates back to sorted order
        eu = euT.T.reshape(T, TILE, BOND_D)
        v = idx >= 0
        edge_update_s[idx[v]] = eu[v]

    aggregated_x = aggX[:N].astype(np.float32)
    aggregated_pos = aggP[:N].astype(np.float32)
    edge_update = np.empty_like(edge_update_s)
    edge_update[order] = edge_update_s
    return aggregated_x, aggregated_pos, edge_update


def kernel(**inputs):
    global LAST_RESULTS
    core_inputs, meta = host_pack(**inputs)
    nc = _build_bass(meta["T"])
    res = bass_utils.run_bass_kernel_spmd(
        nc, core_inputs, core_ids=list(range(NCORES)),
    )
    LAST_RESULTS = res
    return host_merge(res.results, meta)
